# revision 8
# baseline (speedup 1.0000x reference)
"""Trainium2 Bass kernel for LoRALayer: out = 2.0 * (x @ B) @ A.

x: [4, 4096, 4096] f32; A: [8, 4096] f32; B: [4096, 8] f32.
Sharding: data-parallel on the 16384 tokens across 8 cores (2048 each);
A/B replicated. Host-side prep (part of sharding): each core's x-shard is
shipped transposed (xT [4096, 2048]) so the contraction dim lies on SBUF
partitions; B is pre-packed chunk-major; A is pre-scaled by 2.

Per core (all f32, exact), software-pipelined so mm2 of block b-1
interleaves with mm1 of block b (keeps the PE HAM clock warm):
  mm1 (PE): yT[8, 512] += Bp_c[128, 8].T @ xT_c[128, 512]   (32 chunks/block)
  mm2 (PE): out[128, 512] = yT_sub[8, 128].T @ A2[8, 512]    (32 MMs/block)
PSUM->SBUF copies alternate DVE/ACT; input DMAs on the sync HWDGE ring,
output DMAs on the scalar HWDGE ring.
"""

import numpy as np

P = 128
F_IN = 4096
F_OUT = 4096
RANK = 8
N_CORES = 8
SCALING = 2.0
TBLK = 512             # token block (mm1 rhs free dim, f32 max 512)

_CACHE = {}


def _build_nc(T, F_in, F_out, R):
    """Build the single-core Bass program for a T-token shard."""
    from contextlib import ExitStack

    import concourse.mybir as mybir
    import concourse.tile as tile
    from concourse import bacc

    f32 = mybir.dt.float32
    tblk = min(TBLK, T)     # token block (mm1 rhs free dim, f32 max 512)
    CH = F_in // P          # feature chunks (32)
    NB = T // tblk          # token blocks (4)
    NSUB = tblk // P        # 128-token subtiles per block (4)
    NS = F_out // 512       # output column chunks (8)
    CGRP = min(4, CH)       # chunks per input sub-DMA (1MB granularity)
    NDMA = CH // CGRP       # input sub-DMAs per block (8)
    MM2_PER_BLK = NSUB * NS  # 32

    nc = bacc.Bacc("TRN2", target_bir_lowering=False, debug=False)

    xt_d = nc.dram_tensor("xT", [F_in, T], f32, kind="ExternalInput").ap()
    bp_d = nc.dram_tensor("Bp", [P, CH * R], f32, kind="ExternalInput").ap()
    a2_d = nc.dram_tensor("A2", [R, F_out], f32, kind="ExternalInput").ap()
    out_d = nc.dram_tensor("out", [T, F_out], f32, kind="ExternalOutput").ap()

    with tile.TileContext(nc) as tc, ExitStack() as ctx:
        cpool = ctx.enter_context(tc.tile_pool(name="const", bufs=1))
        xtpool = ctx.enter_context(tc.tile_pool(name="xt", bufs=2 * NDMA))
        ytpool = ctx.enter_context(tc.tile_pool(name="yt", bufs=3))
        opool = ctx.enter_context(tc.tile_pool(name="osb", bufs=3))
        y_pp = ctx.enter_context(tc.tile_pool(name="y_ps", bufs=2, space="PSUM"))
        o_pp = ctx.enter_context(tc.tile_pool(name="o_ps", bufs=4, space="PSUM"))

        b_sb = cpool.tile([P, CH * R], f32, tag="b_sb")
        nc.sync.dma_start(b_sb[:], bp_d)
        a_sb = cpool.tile([R, F_out], f32, tag="a_sb")
        nc.sync.dma_start(a_sb[:], a2_d)

        yt_sbs = {}

        def emit_mm2(blk, idx):
            """idx in [0, MM2_PER_BLK): (sub, n) pair for block blk."""
            sub, n = divmod(idx, NS)
            yt_sb, o_sbs = yt_sbs[blk]
            if n == 0:
                o_sbs[sub] = opool.tile(
                    [P, F_out], f32, tag="o_sb", name=f"o_sb_{blk}_{sub}"
                )
            o_sb = o_sbs[sub]
            o_ps = o_pp.tile([P, 512], f32, tag="o_ps")
            nc.tensor.matmul(
                o_ps[:],
                yt_sb[:, sub * P:(sub + 1) * P],
                a_sb[:, n * 512:(n + 1) * 512],
                start=True,
                stop=True,
            )
            if n % 2 == 0:
                nc.scalar.copy(o_sb[:, n * 512:(n + 1) * 512], o_ps[:])
            else:
                nc.vector.tensor_copy(o_sb[:, n * 512:(n + 1) * 512], o_ps[:])
            if n == NS - 1:
                trow = blk * tblk + sub * P
                nc.scalar.dma_start(out_d[trow:trow + P, :], o_sb[:])

        for blk in range(NB + 1):
            xts = []
            if blk < NB:
                t0 = blk * tblk
                src = xt_d[:, t0:t0 + tblk].rearrange("(c p) t -> p c t", p=P)
                for s in range(NDMA):
                    xt_sb = xtpool.tile([P, CGRP, tblk], f32, tag="xt_sb")
                    nc.sync.dma_start(xt_sb[:], src[:, s * CGRP:(s + 1) * CGRP, :])
                    xts.append(xt_sb)
                yt_ps = y_pp.tile([R, tblk], f32, tag="yt_ps")

            # Interleave mm1 of this block 1:1 with mm2 of the previous block.
            n_steps = max(CH if blk < NB else 0, MM2_PER_BLK if blk > 0 else 0)
            for i in range(n_steps):
                if blk > 0 and i < MM2_PER_BLK:
                    emit_mm2(blk - 1, i)
                if blk < NB and i < CH:
                    c = i
                    nc.tensor.matmul(
                        yt_ps[:],
                        b_sb[:, c * R:(c + 1) * R],
                        xts[c // CGRP][:, c % CGRP, :],
                        start=(c == 0),
                        stop=(c == CH - 1),
                    )
            if blk > 0:
                del yt_sbs[blk - 1]
            if blk < NB:
                yt_sb = ytpool.tile([R, tblk], f32, tag="yt_sb")
                nc.vector.tensor_copy(yt_sb[:], yt_ps[:])
                yt_sbs[blk] = (yt_sb, {})

    nc.compile()
    return nc


def _pack_inputs(x2d, A, B, T_shard, F_in, R):
    """Shard x on tokens (shipped transposed); replicate packed B and 2*A."""
    CH = F_in // P
    bp = np.ascontiguousarray(
        B.reshape(CH, P, R).transpose(1, 0, 2).reshape(P, CH * R)
    ).astype(np.float32)
    a2 = np.ascontiguousarray(SCALING * A).astype(np.float32)
    n_shards = x2d.shape[0] // T_shard
    in_maps = []
    for c in range(n_shards):
        xt = np.ascontiguousarray(x2d[c * T_shard:(c + 1) * T_shard].T)
        in_maps.append({"xT": xt, "Bp": bp, "A2": a2})
    return in_maps


def kernel(x, A, B):
    from concourse.bass_utils import run_bass_kernel_spmd

    x = np.asarray(x, dtype=np.float32)
    A = np.asarray(A, dtype=np.float32)
    B = np.asarray(B, dtype=np.float32)
    orig_shape = x.shape
    x2d = x.reshape(-1, F_IN)
    T_shard = x2d.shape[0] // N_CORES

    key = (T_shard, F_IN, F_OUT, RANK)
    if key not in _CACHE:
        _CACHE[key] = _build_nc(T_shard, F_IN, F_OUT, RANK)
    nc = _CACHE[key]

    in_maps = _pack_inputs(x2d, A, B, T_shard, F_IN, RANK)
    res = run_bass_kernel_spmd(nc, in_maps, core_ids=list(range(N_CORES)))
    out = np.concatenate([r["out"] for r in res.results], axis=0)
    return out.reshape(*orig_shape[:-1], F_OUT)


# revision 13
# speedup vs baseline: 2.1809x; 2.1809x over previous
"""Trainium2 Bass kernel for LoRALayer: out = 2.0 * (x @ B) @ A.

x: [4, 4096, 4096] f32; A: [8, 4096] f32; B: [4096, 8] f32.
Sharding: data-parallel on the 16384 tokens across 8 cores (2048 each);
A/B replicated. Host-side prep (part of sharding): each core's x-shard is
shipped transposed (contraction dim on SBUF partitions) and split into
bf16 hi/lo halves (x = hi + lo exactly captures 16 mantissa bits); B and
2*A likewise. bf16 matmuls are ~4x cheaper than fp32 on the PE (single
pass + fast weight load), and the hi/lo compensation keeps ~1e-5 accuracy.

Per core, per 512-token block (f32 PSUM accumulation):
  mm1 chain A: ps_a[16,512] += [B_hi|B_lo]_c.T @ xh_c   (32 chunks; M-packed:
               rows 0-7 = x_hi@B_hi, rows 8-15 = x_hi@B_lo)
  mm1 chain B: ps_b[8,512]  += B_hi_c.T @ xl_c          (x_lo@B_hi)
  y = ps_a[0:8] + ps_a[8:16] + ps_b  (DVE, f32); split y -> y_hi/y_lo bf16,
  pack K-wise as [y_hi; y_hi; y_lo] against A2pk = [A_hi; A_lo; A_hi]:
  mm2: out[128,512] = y_pack_sub[24,128].T @ A2pk[24,512]  (one MM = all 3
       correction terms).
mm2 of block b-1 is interleaved with mm1 of block b (PE density for HAM);
input DMAs ride the sync HWDGE ring, output DMAs the scalar ring; PSUM->SBUF
copies alternate DVE/ACT.
"""

import numpy as np

P = 128
F_IN = 4096
F_OUT = 4096
RANK = 8
N_CORES = 8
SCALING = 2.0
TBLK = 512             # token block (mm1 rhs free dim, max 512 for f32 PSUM out)

_CACHE = {}


def _build_nc(T, F_in, F_out, R):
    """Build the single-core Bass program for a T-token shard."""
    from contextlib import ExitStack

    import concourse.mybir as mybir
    import concourse.tile as tile
    from concourse import bacc

    f32 = mybir.dt.float32
    bf16 = mybir.dt.bfloat16
    tblk = min(TBLK, T)
    CH = F_in // P          # feature chunks (32)
    NB = T // tblk          # token blocks (4)
    NSUB = tblk // P        # 128-token subtiles per block (4)
    NS = F_out // 512       # output column chunks (8)
    CGRP = min(8, CH)       # chunks per input sub-DMA (1MB bf16 granularity)
    NDMA = CH // CGRP       # input sub-DMAs per tensor per block (4)
    MM2_PER_BLK = NSUB * NS  # 32
    RB = 32                  # 32-aligned row blocks (engine partition bases)

    nc = bacc.Bacc("TRN2", target_bir_lowering=False, debug=False)

    xh_d = nc.dram_tensor(
        "xh", [NB, NDMA, P, CGRP * tblk], bf16, kind="ExternalInput"
    ).ap()
    xl_d = nc.dram_tensor(
        "xl", [NB, NDMA, P, CGRP * tblk], bf16, kind="ExternalInput"
    ).ap()
    bpk_d = nc.dram_tensor("Bpk", [P, CH * 2 * RB], bf16, kind="ExternalInput").ap()
    a2pk_d = nc.dram_tensor("A2pk", [3 * RB, F_out], bf16, kind="ExternalInput").ap()
    out_d = nc.dram_tensor("out", [T, F_out], f32, kind="ExternalOutput").ap()

    with tile.TileContext(nc) as tc, ExitStack() as ctx:
        cpool = ctx.enter_context(tc.tile_pool(name="const", bufs=1))
        xtpool = ctx.enter_context(tc.tile_pool(name="xt", bufs=2 * NDMA))
        ytpool = ctx.enter_context(tc.tile_pool(name="yt", bufs=3))
        opool = ctx.enter_context(tc.tile_pool(name="osb", bufs=3))
        y_pp = ctx.enter_context(tc.tile_pool(name="y_ps", bufs=2, space="PSUM"))
        o_pp = ctx.enter_context(tc.tile_pool(name="o_ps", bufs=4, space="PSUM"))

        bpk_sb = cpool.tile([P, CH * 2 * RB], bf16, tag="bpk_sb")
        nc.sync.dma_start(bpk_sb[:], bpk_d)
        apk_sb = cpool.tile([3 * RB, F_out], bf16, tag="apk_sb")
        nc.sync.dma_start(apk_sb[:], a2pk_d)

        blk_state = {}

        def emit_mm2(blk, idx):
            """idx in [0, MM2_PER_BLK): (sub, n) pair for block blk."""
            sub, n = divmod(idx, NS)
            y_pack, o_sbs = blk_state[blk]
            if n == 0:
                o_sbs[sub] = opool.tile(
                    [P, F_out], f32, tag="o_sb", name=f"o_sb_{blk}_{sub}"
                )
            o_sb = o_sbs[sub]
            o_ps = o_pp.tile([P, 512], f32, tag="o_ps")
            nc.tensor.matmul(
                o_ps[:],
                y_pack[:, sub * P:(sub + 1) * P],
                apk_sb[:, n * 512:(n + 1) * 512],
                start=True,
                stop=True,
            )
            if n % 2 == 0:
                nc.scalar.copy(o_sb[:, n * 512:(n + 1) * 512], o_ps[:])
            else:
                nc.vector.tensor_copy(o_sb[:, n * 512:(n + 1) * 512], o_ps[:])
            if n == NS - 1:
                trow = blk * tblk + sub * P
                nc.scalar.dma_start(out_d[trow:trow + P, :], o_sb[:])

        for blk in range(NB + 1):
            xhs, xls = [], []
            if blk < NB:
                for s in range(NDMA):
                    xh_sb = xtpool.tile([P, CGRP, tblk], bf16, tag="xh_sb")
                    nc.sync.dma_start(
                        xh_sb[:].rearrange("p c t -> p (c t)"), xh_d[blk, s]
                    )
                    xhs.append(xh_sb)
                    xl_sb = xtpool.tile([P, CGRP, tblk], bf16, tag="xl_sb")
                    nc.sync.dma_start(
                        xl_sb[:].rearrange("p c t -> p (c t)"), xl_d[blk, s]
                    )
                    xls.append(xl_sb)
                ps_a = y_pp.tile([RB + R, tblk], f32, tag="ps_a")
                ps_b = y_pp.tile([R, tblk], f32, tag="ps_b")

            # Interleave mm1 of this block 1:1 with mm2 of the previous block.
            n_steps = max(CH if blk < NB else 0, MM2_PER_BLK if blk > 0 else 0)
            for i in range(n_steps):
                if blk > 0 and i < MM2_PER_BLK:
                    emit_mm2(blk - 1, i)
                if blk < NB and i < CH:
                    c = i
                    nc.tensor.matmul(
                        ps_a[:],
                        bpk_sb[:, c * 2 * RB:c * 2 * RB + RB + R],
                        xhs[c // CGRP][:, c % CGRP, :],
                        start=(c == 0),
                        stop=(c == CH - 1),
                    )
                    nc.tensor.matmul(
                        ps_b[:],
                        bpk_sb[:, c * 2 * RB:c * 2 * RB + R],
                        xls[c // CGRP][:, c % CGRP, :],
                        start=(c == 0),
                        stop=(c == CH - 1),
                    )
            if blk > 0:
                del blk_state[blk - 1]
            if blk < NB:
                # y = hh + hl + lh (f32), then split into bf16 hi/lo and pack
                # K-wise as [y_hi; y_hi; y_lo] for the one-shot mm2.
                yt32 = ytpool.tile([R, tblk], f32, tag="yt32")
                nc.vector.tensor_copy(yt32[:], ps_a[:R, :])
                nc.vector.tensor_add(yt32[:], yt32[:], ps_a[RB:RB + R, :])
                nc.vector.tensor_add(yt32[:], yt32[:], ps_b[:])
                y_pack = ytpool.tile([3 * RB, tblk], bf16, tag="y_pack")
                nc.gpsimd.memset(y_pack[:], 0.0)
                nc.vector.tensor_copy(y_pack[:R, :], yt32[:])               # y_hi
                nc.vector.tensor_copy(y_pack[RB:RB + R, :], y_pack[:R, :])  # dup
                y_hi32 = ytpool.tile([R, tblk], f32, tag="y_hi32")
                nc.vector.tensor_copy(y_hi32[:], y_pack[:R, :])             # f32
                nc.vector.tensor_sub(y_pack[2 * RB:2 * RB + R, :], yt32[:], y_hi32[:])
                blk_state[blk] = (y_pack, {})

    nc.compile()
    return nc


def _pack_inputs(x2d, A, B, T_shard, F_in, R):
    """Shard x on tokens (transposed + bf16 hi/lo split); replicate B/A packs."""
    import ml_dtypes

    bf16 = ml_dtypes.bfloat16
    CH = F_in // P

    def split(m):
        hi = m.astype(bf16)
        lo = (m - hi.astype(np.float32)).astype(bf16)
        return hi, lo

    RB = 32
    R = B.shape[1]
    Bh, Bl = split(B.astype(np.float32))
    # chunk-major pack, 32-aligned: per chunk c of 2*RB cols:
    #   [0:R]=B_hi, [RB:RB+R]=B_lo, rest zero
    bpk = np.zeros((CH, P, 2 * RB), dtype=Bh.dtype)
    bpk[:, :, :R] = Bh.reshape(CH, P, R)
    bpk[:, :, RB:RB + R] = Bl.reshape(CH, P, R)
    bpk = np.ascontiguousarray(bpk.transpose(1, 0, 2).reshape(P, CH * 2 * RB))

    A2 = (SCALING * A).astype(np.float32)
    Ah, Al = split(A2)
    a2pk = np.zeros((3 * RB, A2.shape[1]), dtype=Ah.dtype)
    a2pk[:R] = Ah
    a2pk[RB:RB + R] = Al
    a2pk[2 * RB:2 * RB + R] = Ah
    a2pk = np.ascontiguousarray(a2pk)

    # device-DMA-friendly pack: [NB, NDMA, P, CGRP*tblk] so each sub-DMA
    # reads one contiguous per-partition run.
    T = T_shard
    tblk = min(TBLK, T)
    NB = T // tblk
    CGRP = min(8, CH)
    NDMA = CH // CGRP

    def pack(m):
        a = m.reshape(NDMA, CGRP, P, NB, tblk)
        a = a.transpose(3, 0, 2, 1, 4)
        return np.ascontiguousarray(a.reshape(NB, NDMA, P, CGRP * tblk))

    n_shards = x2d.shape[0] // T_shard
    in_maps = []
    for c in range(n_shards):
        xt = np.ascontiguousarray(x2d[c * T_shard:(c + 1) * T_shard].T)
        xh, xl = split(xt)
        in_maps.append(
            {"xh": pack(xh), "xl": pack(xl), "Bpk": bpk, "A2pk": a2pk}
        )
    return in_maps


def kernel(x, A, B):
    from concourse.bass_utils import run_bass_kernel_spmd

    x = np.asarray(x, dtype=np.float32)
    A = np.asarray(A, dtype=np.float32)
    B = np.asarray(B, dtype=np.float32)
    orig_shape = x.shape
    x2d = x.reshape(-1, F_IN)
    T_shard = x2d.shape[0] // N_CORES

    key = (T_shard, F_IN, F_OUT, RANK)
    if key not in _CACHE:
        _CACHE[key] = _build_nc(T_shard, F_IN, F_OUT, RANK)
    nc = _CACHE[key]

    in_maps = _pack_inputs(x2d, A, B, T_shard, F_IN, RANK)
    res = run_bass_kernel_spmd(nc, in_maps, core_ids=list(range(N_CORES)))
    out = np.concatenate([r["out"] for r in res.results], axis=0)
    return out.reshape(*orig_shape[:-1], F_OUT)


# revision 14
# speedup vs baseline: 2.4830x; 1.1385x over previous
"""Trainium2 Bass kernel for LoRALayer: out = 2.0 * (x @ B) @ A.

x: [4, 4096, 4096] f32; A: [8, 4096] f32; B: [4096, 8] f32.
Sharding: data-parallel on the 16384 tokens across 8 cores (2048 each);
A/B replicated. Host-side prep (part of sharding): each core's x-shard is
shipped transposed (contraction dim on SBUF partitions) and split into
bf16 hi/lo halves (x = hi + lo exactly captures 16 mantissa bits); B and
2*A likewise. bf16 matmuls are ~4x cheaper than fp32 on the PE (single
pass + fast weight load), and the hi/lo compensation keeps ~1e-5 accuracy.

Per core, per 512-token block (f32 PSUM accumulation):
  mm1 chain A: ps_a[16,512] += [B_hi|B_lo]_c.T @ xh_c   (32 chunks; M-packed:
               rows 0-7 = x_hi@B_hi, rows 8-15 = x_hi@B_lo)
  mm1 chain B: ps_b[8,512]  += B_hi_c.T @ xl_c          (x_lo@B_hi)
  y = ps_a[0:8] + ps_a[8:16] + ps_b  (DVE, f32); split y -> y_hi/y_lo bf16,
  pack K-wise as [y_hi; y_hi; y_lo] against A2pk = [A_hi; A_lo; A_hi]:
  mm2: out[128,512] = y_pack_sub[24,128].T @ A2pk[24,512]  (one MM = all 3
       correction terms).
mm2 of block b-1 is interleaved with mm1 of block b (PE density for HAM);
input DMAs ride the sync HWDGE ring, output DMAs the scalar ring; PSUM->SBUF
copies alternate DVE/ACT.
"""

import numpy as np

P = 128
F_IN = 4096
F_OUT = 4096
RANK = 8
N_CORES = 8
SCALING = 2.0
TBLK = 256             # token block (mm1 rhs free dim, max 512 for f32 PSUM out)

_CACHE = {}


def _build_nc(T, F_in, F_out, R):
    """Build the single-core Bass program for a T-token shard."""
    from contextlib import ExitStack

    import concourse.mybir as mybir
    import concourse.tile as tile
    from concourse import bacc

    f32 = mybir.dt.float32
    bf16 = mybir.dt.bfloat16
    tblk = min(TBLK, T)
    CH = F_in // P          # feature chunks (32)
    NB = T // tblk          # token blocks (4)
    NSUB = tblk // P        # 128-token subtiles per block (4)
    NS = F_out // 512       # output column chunks (8)
    CGRP = min(8, CH)       # chunks per input sub-DMA (1MB bf16 granularity)
    NDMA = CH // CGRP       # input sub-DMAs per tensor per block (4)
    MM2_PER_BLK = NSUB * NS  # 32
    RB = 32                  # 32-aligned row blocks (engine partition bases)

    nc = bacc.Bacc("TRN2", target_bir_lowering=False, debug=False)

    xh_d = nc.dram_tensor(
        "xh", [NB, NDMA, P, CGRP * tblk], bf16, kind="ExternalInput"
    ).ap()
    xl_d = nc.dram_tensor(
        "xl", [NB, NDMA, P, CGRP * tblk], bf16, kind="ExternalInput"
    ).ap()
    bpk_d = nc.dram_tensor("Bpk", [P, CH * 2 * RB], bf16, kind="ExternalInput").ap()
    a2pk_d = nc.dram_tensor("A2pk", [3 * RB, F_out], bf16, kind="ExternalInput").ap()
    out_d = nc.dram_tensor("out", [T, F_out], f32, kind="ExternalOutput").ap()

    with tile.TileContext(nc) as tc, ExitStack() as ctx:
        cpool = ctx.enter_context(tc.tile_pool(name="const", bufs=1))
        xtpool = ctx.enter_context(tc.tile_pool(name="xt", bufs=2 * NDMA))
        ytpool = ctx.enter_context(tc.tile_pool(name="yt", bufs=3))
        opool = ctx.enter_context(tc.tile_pool(name="osb", bufs=3))
        y_pp = ctx.enter_context(tc.tile_pool(name="y_ps", bufs=2, space="PSUM"))
        o_pp = ctx.enter_context(tc.tile_pool(name="o_ps", bufs=4, space="PSUM"))

        bpk_sb = cpool.tile([P, CH * 2 * RB], bf16, tag="bpk_sb")
        nc.sync.dma_start(bpk_sb[:], bpk_d)
        apk_sb = cpool.tile([3 * RB, F_out], bf16, tag="apk_sb")
        nc.sync.dma_start(apk_sb[:], a2pk_d)

        blk_state = {}

        def emit_mm2(blk, idx):
            """idx in [0, MM2_PER_BLK): (sub, n) pair for block blk."""
            sub, n = divmod(idx, NS)
            y_pack, o_sbs = blk_state[blk]
            if n == 0:
                o_sbs[sub] = opool.tile(
                    [P, F_out], f32, tag="o_sb", name=f"o_sb_{blk}_{sub}"
                )
            o_sb = o_sbs[sub]
            o_ps = o_pp.tile([P, 512], f32, tag="o_ps")
            nc.tensor.matmul(
                o_ps[:],
                y_pack[:, sub * P:(sub + 1) * P],
                apk_sb[:, n * 512:(n + 1) * 512],
                start=True,
                stop=True,
            )
            if n % 2 == 0:
                nc.scalar.copy(o_sb[:, n * 512:(n + 1) * 512], o_ps[:])
            else:
                nc.vector.tensor_copy(o_sb[:, n * 512:(n + 1) * 512], o_ps[:])
            if n == NS - 1:
                trow = blk * tblk + sub * P
                nc.scalar.dma_start(out_d[trow:trow + P, :], o_sb[:])

        for blk in range(NB + 1):
            xhs, xls = [], []
            if blk < NB:
                for s in range(NDMA):
                    xh_sb = xtpool.tile([P, CGRP, tblk], bf16, tag="xh_sb")
                    nc.sync.dma_start(
                        xh_sb[:].rearrange("p c t -> p (c t)"), xh_d[blk, s]
                    )
                    xhs.append(xh_sb)
                    xl_sb = xtpool.tile([P, CGRP, tblk], bf16, tag="xl_sb")
                    nc.sync.dma_start(
                        xl_sb[:].rearrange("p c t -> p (c t)"), xl_d[blk, s]
                    )
                    xls.append(xl_sb)
                ps_a = y_pp.tile([RB + R, tblk], f32, tag="ps_a")
                ps_b = y_pp.tile([R, tblk], f32, tag="ps_b")

            # Interleave mm1 of this block 1:1 with mm2 of the previous block.
            n_steps = max(CH if blk < NB else 0, MM2_PER_BLK if blk > 0 else 0)
            for i in range(n_steps):
                if blk > 0 and i < MM2_PER_BLK:
                    emit_mm2(blk - 1, i)
                if blk < NB and i < CH:
                    c = i
                    nc.tensor.matmul(
                        ps_a[:],
                        bpk_sb[:, c * 2 * RB:c * 2 * RB + RB + R],
                        xhs[c // CGRP][:, c % CGRP, :],
                        start=(c == 0),
                        stop=(c == CH - 1),
                    )
                    nc.tensor.matmul(
                        ps_b[:],
                        bpk_sb[:, c * 2 * RB:c * 2 * RB + R],
                        xls[c // CGRP][:, c % CGRP, :],
                        start=(c == 0),
                        stop=(c == CH - 1),
                    )
            if blk > 0:
                del blk_state[blk - 1]
            if blk < NB:
                # y = hh + hl + lh (f32), then split into bf16 hi/lo and pack
                # K-wise as [y_hi; y_hi; y_lo] for the one-shot mm2.
                yt32 = ytpool.tile([R, tblk], f32, tag="yt32")
                nc.vector.tensor_copy(yt32[:], ps_a[:R, :])
                nc.vector.tensor_add(yt32[:], yt32[:], ps_a[RB:RB + R, :])
                nc.vector.tensor_add(yt32[:], yt32[:], ps_b[:])
                y_pack = ytpool.tile([3 * RB, tblk], bf16, tag="y_pack")
                nc.gpsimd.memset(y_pack[:], 0.0)
                nc.vector.tensor_copy(y_pack[:R, :], yt32[:])               # y_hi
                nc.vector.tensor_copy(y_pack[RB:RB + R, :], y_pack[:R, :])  # dup
                y_hi32 = ytpool.tile([R, tblk], f32, tag="y_hi32")
                nc.vector.tensor_copy(y_hi32[:], y_pack[:R, :])             # f32
                nc.vector.tensor_sub(y_pack[2 * RB:2 * RB + R, :], yt32[:], y_hi32[:])
                blk_state[blk] = (y_pack, {})

    nc.compile()
    return nc


def _pack_inputs(x2d, A, B, T_shard, F_in, R):
    """Shard x on tokens (transposed + bf16 hi/lo split); replicate B/A packs."""
    import ml_dtypes

    bf16 = ml_dtypes.bfloat16
    CH = F_in // P

    def split(m):
        hi = m.astype(bf16)
        lo = (m - hi.astype(np.float32)).astype(bf16)
        return hi, lo

    RB = 32
    R = B.shape[1]
    Bh, Bl = split(B.astype(np.float32))
    # chunk-major pack, 32-aligned: per chunk c of 2*RB cols:
    #   [0:R]=B_hi, [RB:RB+R]=B_lo, rest zero
    bpk = np.zeros((CH, P, 2 * RB), dtype=Bh.dtype)
    bpk[:, :, :R] = Bh.reshape(CH, P, R)
    bpk[:, :, RB:RB + R] = Bl.reshape(CH, P, R)
    bpk = np.ascontiguousarray(bpk.transpose(1, 0, 2).reshape(P, CH * 2 * RB))

    A2 = (SCALING * A).astype(np.float32)
    Ah, Al = split(A2)
    a2pk = np.zeros((3 * RB, A2.shape[1]), dtype=Ah.dtype)
    a2pk[:R] = Ah
    a2pk[RB:RB + R] = Al
    a2pk[2 * RB:2 * RB + R] = Ah
    a2pk = np.ascontiguousarray(a2pk)

    # device-DMA-friendly pack: [NB, NDMA, P, CGRP*tblk] so each sub-DMA
    # reads one contiguous per-partition run.
    T = T_shard
    tblk = min(TBLK, T)
    NB = T // tblk
    CGRP = min(8, CH)
    NDMA = CH // CGRP

    def pack(m):
        a = m.reshape(NDMA, CGRP, P, NB, tblk)
        a = a.transpose(3, 0, 2, 1, 4)
        return np.ascontiguousarray(a.reshape(NB, NDMA, P, CGRP * tblk))

    n_shards = x2d.shape[0] // T_shard
    in_maps = []
    for c in range(n_shards):
        xt = np.ascontiguousarray(x2d[c * T_shard:(c + 1) * T_shard].T)
        xh, xl = split(xt)
        in_maps.append(
            {"xh": pack(xh), "xl": pack(xl), "Bpk": bpk, "A2pk": a2pk}
        )
    return in_maps


def kernel(x, A, B):
    from concourse.bass_utils import run_bass_kernel_spmd

    x = np.asarray(x, dtype=np.float32)
    A = np.asarray(A, dtype=np.float32)
    B = np.asarray(B, dtype=np.float32)
    orig_shape = x.shape
    x2d = x.reshape(-1, F_IN)
    T_shard = x2d.shape[0] // N_CORES

    key = (T_shard, F_IN, F_OUT, RANK)
    if key not in _CACHE:
        _CACHE[key] = _build_nc(T_shard, F_IN, F_OUT, RANK)
    nc = _CACHE[key]

    in_maps = _pack_inputs(x2d, A, B, T_shard, F_IN, RANK)
    res = run_bass_kernel_spmd(nc, in_maps, core_ids=list(range(N_CORES)))
    out = np.concatenate([r["out"] for r in res.results], axis=0)
    return out.reshape(*orig_shape[:-1], F_OUT)
